# revision 13
# baseline (speedup 1.0000x reference)
"""Trainium2 Bass kernel for nn_AdaptiveGeometryModule.

Data-parallel over batch across 8 NeuronCores. Key algebraic facts used:
  - The seq-len-1 multi-head attention is the identity on v, so
      refined = balanced @ (Wv @ proj_w) + (bv @ proj_w + proj_b)
    with Wv = qkv_w[:, 2D:3D], bv = qkv_b[2D:3D].
  - balanced = (softmax(-dist/temp) @ anch) * mlp_weight, so with
    U = exp(logits), rinv = 1/rowsum(U), w = sigmoid(mlp),
      refined = ([U * rinv * w | 1] @ [anch @ Wv @ proj_w ; b_comb])
  - center_loss rows = ||f||^2 - 2 f.refined + ||refined||^2, with
      f.refined = sum_a U'[a] * (f @ McatT)[a]   (extra matmul columns)
      ||refined||^2 = U' @ (Mcat Mcat^T) @ U'^T  (tiny Gram quadratic form)
No collectives needed: each core emits a scalar loss partial; host sums.
"""

import numpy as np
import ml_dtypes

import concourse.bass as bass
import concourse.mybir as mybir
import concourse.tile as tile
from concourse.bass_utils import run_bass_kernel_spmd

F32 = mybir.dt.float32
F32R = mybir.dt.float32r
BF16 = mybir.dt.bfloat16
AX = mybir.AxisListType
OP = mybir.AluOpType
ACTF = mybir.ActivationFunctionType

B, D, A, H = 16384, 512, 10, 8
NCORES = 8
BL = B // NCORES          # 2048 rows per core
NT = BL // 128            # 16 tiles of 128 rows
NCH = NT // 4             # 4 chunks of 512 rows
DC = D // 128             # 4 d-chunks
NCOLS = A + A + 1         # 21: [-2*anch | McatT(11)] columns

_CACHE = {}


def _split_waits(nc, maxw=1):
    """walrus on this image rejects instructions with more than a couple
    of inline sync-wait commands (Tile's tail drain aggregates one per
    engine domain). Split excess waits onto same-engine NOPs placed
    immediately before the instruction — identical semantics, since the
    engine stalls at the NOP's wait either way."""
    f = nc.m.functions[0]
    cur = nc.cur_bb.bb
    for bb in f.blocks:
        il = list(bb.instructions)
        need = any(i.sync_info and i.sync_info.on_wait
                   and len(i.sync_info.on_wait) > maxw for i in il)
        if not need:
            continue
        out = []
        for inst in il:
            si = inst.sync_info
            waits = list(si.on_wait) if si and si.on_wait else []
            if len(waits) > maxw:
                extra, keep = waits[:-maxw], waits[-maxw:]
                for i in range(0, len(extra), maxw):
                    b = nc.engines[inst.engine].nop(hint="waitsplit")
                    n = b.ins
                    cl = cur.instructions
                    assert cl[-1].name == n.name
                    cur.instructions = cl[:-1]
                    n.sync_info = mybir.SyncInfo(
                        on_wait=extra[i:i + maxw], on_update=[])
                    out.append(n)
                inst.sync_info = mybir.SyncInfo(
                    on_wait=keep,
                    on_update=list(si.on_update)
                    if si and si.on_update else [])
            out.append(inst)
        bb.instructions = out


def build_nc():
    nc = bass.Bass("TRN2", target_bir_lowering=False, debug=False,
                   num_devices=NCORES)

    # ---- dram parameters (per-core shard + replicated constants) ----
    d_feat = nc.dram_tensor("features", [BL, D], F32, kind="ExternalInput")
    d_w1 = nc.dram_tensor("c_w1", [D, 256], BF16, kind="ExternalInput")
    d_w2 = nc.dram_tensor("c_w2", [128, 2], BF16, kind="ExternalInput")
    d_b1 = nc.dram_tensor("c_b1", [128, 2], F32, kind="ExternalInput")
    d_b2 = nc.dram_tensor("c_b2", [128, 1], F32, kind="ExternalInput")
    d_anchcat = nc.dram_tensor("c_anchcat", [D, NCOLS], BF16,
                               kind="ExternalInput")
    d_na2 = nc.dram_tensor("c_na2cat", [1, NCOLS], F32, kind="ExternalInput")
    d_mcat = nc.dram_tensor("c_mcat", [A + 1, D], BF16, kind="ExternalInput")
    d_gcat = nc.dram_tensor("c_gcat", [A + 1, A + 1], BF16,
                            kind="ExternalInput")
    d_id16 = nc.dram_tensor("c_id16", [128, 128], BF16, kind="ExternalInput")
    d_id32 = nc.dram_tensor("c_id32", [128, 128], F32, kind="ExternalInput")
    d_onesr = nc.dram_tensor("c_onesr", [1, 128], F32, kind="ExternalInput")
    d_onesc = nc.dram_tensor("c_onesc", [128, 1], F32, kind="ExternalInput")
    d_ones10 = nc.dram_tensor("c_ones10", [A, 1], F32, kind="ExternalInput")
    d_mask = nc.dram_tensor("c_mask", [A, A], F32, kind="ExternalInput")
    d_arT = nc.dram_tensor("c_arT", [D, A], F32, kind="ExternalInput")
    d_arTm2 = nc.dram_tensor("c_arTm2", [D, A], F32, kind="ExternalInput")
    d_anch = nc.dram_tensor("anchors", [A, D], F32, kind="ExternalInput")
    d_clw = nc.dram_tensor("center_loss_weight", [1, 1], F32,
                           kind="ExternalInput")

    d_ref = nc.dram_tensor("refined", [BL, D], F32, kind="ExternalOutput")
    d_att = nc.dram_tensor("attention", [BL, A], F32, kind="ExternalOutput")
    d_loss = nc.dram_tensor("loss", [1, 1], F32, kind="ExternalOutput")

    from contextlib import ExitStack
    with tile.TileContext(nc) as tc, ExitStack() as ctx:
        # ---------------- persistent SBUF pools ----------------
        pc = ctx.enter_context(tc.tile_pool(name="consts", bufs=1))
        p_f16 = ctx.enter_context(tc.tile_pool(name="f16", bufs=NT))
        p_ftT = ctx.enter_context(tc.tile_pool(name="ftT", bufs=NT))
        p_rel = ctx.enter_context(tc.tile_pool(name="relu", bufs=2 * NCH))
        p_ref = ctx.enter_context(tc.tile_pool(name="refsb", bufs=NT))
        p_sm = ctx.enter_context(tc.tile_pool(name="smalls", bufs=NT))
        p_chn = ctx.enter_context(tc.tile_pool(name="chunks", bufs=NCH))
        p_scr = ctx.enter_context(tc.tile_pool(name="scratch", bufs=2))

        # PSUM pools (8 banks total budget)
        ps_tr = ctx.enter_context(
            tc.tile_pool(name="ps_tr", bufs=2, space="PSUM"))
        ps_hid = ctx.enter_context(
            tc.tile_pool(name="ps_hid", bufs=1, space="PSUM"))
        ps_ref = ctx.enter_context(
            tc.tile_pool(name="ps_ref", bufs=2, space="PSUM"))
        ps_dot = ctx.enter_context(
            tc.tile_pool(name="ps_dot", bufs=2, space="PSUM"))
        ps_sm = ctx.enter_context(
            tc.tile_pool(name="ps_sm", bufs=1, space="PSUM"))

        def csb(dram, shape, dt, name):
            t = pc.tile(shape, dt, tag=name)
            nc.sync.dma_start(t[:], dram[:, :])
            return t

        w1sb = [csb(d_w1[128 * c:128 * (c + 1), :], [128, 256], BF16,
                    f"w1_{c}") for c in range(DC)]
        w2sb = csb(d_w2, [128, 2], BF16, "w2")
        b1sb = csb(d_b1, [128, 2], F32, "b1")
        b2sb = csb(d_b2, [128, 1], F32, "b2")
        acsb = [csb(d_anchcat[128 * c:128 * (c + 1), :], [128, NCOLS], BF16,
                    f"ac_{c}") for c in range(DC)]
        na2sb = csb(d_na2, [1, NCOLS], F32, "na2")
        mcatsb = csb(d_mcat, [A + 1, D], BF16, "mcat")
        gcatsb = csb(d_gcat, [A + 1, A + 1], BF16, "gcat")
        id16 = csb(d_id16, [128, 128], BF16, "id16")
        id32 = csb(d_id32, [128, 128], F32, "id32")
        onesr = csb(d_onesr, [1, 128], F32, "onesr")
        onesc = csb(d_onesc, [128, 1], F32, "onesc")
        ones10 = csb(d_ones10, [A, 1], F32, "ones10")
        masksb = csb(d_mask, [A, A], F32, "mask")
        arTsb = [csb(d_arT[128 * c:128 * (c + 1), :], [128, A], F32,
                     f"arT_{c}") for c in range(DC)]
        arTm2sb = [csb(d_arTm2[128 * c:128 * (c + 1), :], [128, A], F32,
                       f"arTm2_{c}") for c in range(DC)]
        anchsb = csb(d_anch, [A, D], F32, "anchrow")
        clwsb = csb(d_clw, [1, 1], F32, "clw")

        css_all = pc.tile([128, NT], F32, tag="css_all")

        # persistent U'cat tiles; ones column preset once
        uc_all = []
        for t in range(NT):
            uc = p_sm.tile([128, A + 1], F32, tag="uc")
            nc.gpsimd.memset(uc[:, A:A + 1], 1.0)
            uc_all.append(uc)

        # ---------------- load feature tiles (cast f32 -> bf16) ------------
        f16 = []
        for t in range(NT):
            ft = p_f16.tile([128, D], BF16, tag="f16")
            nc.gpsimd.dma_start(ft[:], d_feat[128 * t:128 * (t + 1), :])
            f16.append(ft)

        featT = {}   # (c, k) -> [128, 512] bf16
        norm2 = []   # per chunk [128, 4]
        csss = []

        for k in range(NCH):
            tiles = [4 * k + j for j in range(4)]
            # -- norm^2 via ACT square + accumulate --
            n2 = p_chn.tile([128, 4], F32, tag="norm2")
            norm2.append(n2)
            for j, t in enumerate(tiles):
                scr = p_scr.tile([128, D], BF16, tag="sqscr")
                nc.scalar.activation(scr[:], f16[t][:], ACTF.Square,
                                     accum_out=n2[:, j:j + 1])

            # -- transposes: featT[c,k][128d, 512r], c-pairs through 2 banks
            for c in range(DC):
                ptr = ps_tr.tile([128, 512], BF16, tag="tr")
                for j, t in enumerate(tiles):
                    nc.tensor.transpose(
                        ptr[:, 128 * j:128 * (j + 1)],
                        f16[t][:, 128 * c:128 * (c + 1)], id16[:])
                fT = p_ftT.tile([128, 512], BF16, tag="ftT")
                if c % 2 == 0:
                    nc.vector.tensor_copy(fT[:], ptr[:])
                else:
                    nc.scalar.activation(fT[:], ptr[:], ACTF.Copy)
                featT[(c, k)] = fT

            # -- MLP hiddenT: [128ch, 512r] x2 halves; relu evac w/ bias --
            relus = []
            for h in range(2):
                ph = ps_hid.tile([128, 512], F32, tag="hid")
                for c in range(DC):
                    nc.tensor.matmul(
                        ph[:], w1sb[c][:, 128 * h:128 * (h + 1)],
                        featT[(c, k)][:], start=(c == 0), stop=(c == DC - 1))
                rl = p_rel.tile([128, 512], BF16, tag="relu")
                nc.scalar.activation(rl[:], ph[:], ACTF.Relu,
                                     bias=b1sb[:, h:h + 1])
                relus.append(rl)

            # -- wl = relu @ w2 -> [1, 512] -> repack -> sigmoid --
            pwl = ps_sm.tile([1, 512], F32, tag="sm")
            for h in range(2):
                nc.tensor.matmul(pwl[:], w2sb[:, h:h + 1], relus[h][:],
                                 start=(h == 0), stop=(h == 1))
            wlsb = p_chn.tile([1, 512], F32, tag="wlsb")
            nc.vector.tensor_copy(wlsb[:], pwl[:])
            pwt = ps_sm.tile([128, 4], F32, tag="sm")
            for j in range(4):
                nc.tensor.transpose(pwt[:, j:j + 1],
                                    wlsb[0:1, 128 * j:128 * (j + 1)],
                                    id32[0:1, 0:1])
            wtil = p_chn.tile([128, 4], F32, tag="wtil")
            nc.scalar.activation(wtil[:], pwt[:], ACTF.Sigmoid,
                                 bias=b2sb[:, 0:1])

            # -- temp = sqrt(norm2), rtemp = 1/temp (batched per chunk) --
            tmp = p_chn.tile([128, 4], F32, tag="tempb")
            nc.scalar.activation(tmp[:], n2[:], ACTF.Sqrt)
            rtemp = p_chn.tile([128, 4], F32, tag="rtemp")
            nc.vector.reciprocal(rtemp[:], tmp[:])

            # -- dots (row layout): one PSUM [128, 4*21] per chunk --
            pdall = ps_dot.tile([128, 4 * NCOLS], F32, tag="dots")
            pdots = []
            for j, t in enumerate(tiles):
                pd = pdall[:, NCOLS * j:NCOLS * (j + 1)]
                # open group with the na^2 rank-1 fold, then accumulate dots
                nc.tensor.matmul(pd, onesr[:], na2sb[:], start=True,
                                 stop=False, skip_group_check=True)
                for c in range(DC):
                    nc.tensor.matmul(
                        pd, featT[(c, k)][:, 128 * j:128 * (j + 1)],
                        acsb[c][:], start=False, stop=(c == DC - 1),
                        skip_group_check=True)
                pdots.append(pd)

            rowsum = p_chn.tile([128, 4], F32, tag="rowsum")
            Us = []
            for j, t in enumerate(tiles):
                pd = pdots[j]
                dist = p_sm.tile([128, A], F32, tag="dist")
                nc.scalar.activation(dist[:], pd[:, 0:A], ACTF.Sqrt,
                                     bias=n2[:, j:j + 1])
                logit = p_sm.tile([128, A], F32, tag="logit")
                nc.vector.tensor_scalar(logit[:], dist[:], rtemp[:, j:j + 1],
                                        -1.0, OP.mult, OP.mult)
                U = p_sm.tile([128, A], F32, tag="U")
                nc.scalar.activation(U[:], logit[:], ACTF.Exp,
                                     accum_out=rowsum[:, j:j + 1])
                Us.append(U)
            rinv = p_chn.tile([128, 4], F32, tag="rinv")
            nc.vector.reciprocal(rinv[:], rowsum[:])

            # -- U' = U*rinv*wtil | 1 ; attention out = U*rinv --
            puT = ps_sm.tile([A + 1, 512], F32, tag="sm")
            ucs = []
            for j, t in enumerate(tiles):
                att = p_sm.tile([128, A], F32, tag="att")
                nc.vector.tensor_scalar(att[:], Us[j][:], rinv[:, j:j + 1],
                                        None, OP.mult)
                nc.sync.dma_start(d_att[128 * t:128 * (t + 1), :], att[:])
                uc = uc_all[t]
                nc.vector.tensor_scalar(uc[:, 0:A], Us[j][:],
                                        rinv[:, j:j + 1], wtil[:, j:j + 1],
                                        OP.mult, OP.mult)
                ucs.append(uc)
                nc.tensor.transpose(puT[:, 128 * j:128 * (j + 1)], uc[:],
                                    id32[:])
            uT = p_chn.tile([A + 1, 512], BF16, tag="uT")
            nc.vector.tensor_copy(uT[:], puT[:])

            # -- refined + V + center pieces --
            fr = p_chn.tile([128, 4], F32, tag="fr")
            qq = p_chn.tile([128, 4], F32, tag="qq")
            for j, t in enumerate(tiles):
                pr = ps_ref.tile([128, D], F32, tag="ref")
                nc.tensor.matmul(pr[:],
                                 uT[:, 128 * j:128 * (j + 1)],
                                 mcatsb[:], start=True, stop=True)
                pv = ps_sm.tile([128, A + 1], F32, tag="sm")
                nc.tensor.matmul(pv[:], uT[:, 128 * j:128 * (j + 1)],
                                 gcatsb[:], start=True, stop=True)
                scr = p_scr.tile([128, A + 1], F32, tag="s11")
                nc.vector.tensor_tensor(scr[:], ucs[j][:],
                                        pdots[j][:, A:NCOLS], OP.mult)
                nc.vector.tensor_reduce(fr[:, j:j + 1], scr[:], AX.X, OP.add)
                scr2 = p_scr.tile([128, A + 1], F32, tag="s11b")
                nc.vector.tensor_tensor(scr2[:], ucs[j][:], pv[:], OP.mult)
                nc.vector.tensor_reduce(qq[:, j:j + 1], scr2[:], AX.X,
                                        OP.add)
                rsb = p_ref.tile([128, D], F32, tag="refsb")
                if j % 2 == 0:
                    nc.vector.tensor_copy(rsb[:], pr[:])
                else:
                    nc.scalar.activation(rsb[:], pr[:], ACTF.Copy)
                nc.sync.dma_start(d_ref[128 * t:128 * (t + 1), :], rsb[:])

            # css = norm2 - 2*fr + q
            t1 = p_chn.tile([128, 4], F32, tag="t1")
            nc.vector.tensor_scalar(t1[:], fr[:], -2.0, None, OP.mult)
            nc.vector.tensor_tensor(t1[:], t1[:], qq[:], OP.add)
            nc.vector.tensor_tensor(css_all[:, 4 * k:4 * (k + 1)], t1[:],
                                    n2[:], OP.add)
            csss.append(k)

        # ---------------- loss tail ----------------
        csum = p_sm.tile([128, 1], F32, tag="csum")
        nc.vector.tensor_reduce(csum[:], css_all[:], AX.X, OP.add)
        pcs = ps_sm.tile([1, 1], F32, tag="sm")
        nc.tensor.matmul(pcs[:], csum[:], onesc[:], start=True, stop=True)

        # diversity on-device
        na2r = p_sm.tile([A, 1], F32, tag="na2r")
        scrA = p_scr.tile([A, D], F32, tag="scrA")
        nc.scalar.activation(scrA[:], anchsb[:], ACTF.Square,
                             accum_out=na2r[:])
        pna = ps_sm.tile([1, A], F32, tag="sm")
        nc.tensor.transpose(pna[:], na2r[:], id32[0:A, 0:A])
        na2row = p_sm.tile([1, A], F32, tag="na2row")
        nc.vector.tensor_copy(na2row[:], pna[:])
        pg = ps_sm.tile([A, A], F32, tag="sm")
        nc.tensor.matmul(pg[:], onesr[:, 0:A], na2row[:], start=True,
                         stop=False)
        for c in range(DC):
            nc.tensor.matmul(pg[:], arTm2sb[c][:], arTsb[c][:],
                             start=False, stop=(c == DC - 1))
        pd2 = p_sm.tile([A, A], F32, tag="pd2")
        nc.vector.tensor_scalar(pd2[:], pg[:], na2r[:], 0.0, OP.add, OP.max)
        pdm = p_sm.tile([A, A], F32, tag="pdm")
        nc.scalar.activation(pdm[:], pd2[:], ACTF.Sqrt)
        scrm = p_scr.tile([A, A], F32, tag="scrm")
        pdsum = p_sm.tile([A, 1], F32, tag="pdsum")
        nc.vector.tensor_tensor(scrm[:], pdm[:], masksb[:], OP.mult)
        nc.vector.tensor_reduce(pdsum[:], scrm[:], AX.X, OP.add)
        ppd = ps_sm.tile([1, 1], F32, tag="sm")
        nc.tensor.matmul(ppd[:], pdsum[:], ones10[:], start=True, stop=True)

        lossA = p_sm.tile([1, 1], F32, tag="lossA")
        nc.vector.tensor_scalar(lossA[:], pcs[:], clwsb[:, 0:1], 1.0 / B,
                                OP.mult, OP.mult)
        lossB = p_sm.tile([1, 1], F32, tag="lossB")
        nc.vector.tensor_scalar(lossB[:], ppd[:], -0.1 / 45.0 / NCORES, None,
                                OP.mult)
        lsb = p_sm.tile([1, 1], F32, tag="lsb")
        nc.vector.tensor_tensor(lsb[:], lossA[:], lossB[:], OP.add)
        nc.sync.dma_start(d_loss[:, :], lsb[:])

    _split_waits(nc)
    return nc


def host_consts(anchors, pos_embedding, qkv_w, qkv_b, proj_w, proj_b,
                w1, b1, w2, b2, center_loss_weight):
    f32, bf16 = np.float32, ml_dtypes.bfloat16
    anch = (anchors + pos_embedding[0]).astype(f32)          # [A, D]
    Wv = qkv_w[:, 2 * D:3 * D].astype(f32)
    bv = qkv_b[2 * D:3 * D].astype(f32)
    Mcomb = (anch @ Wv @ proj_w).astype(f32)                 # [A, D]
    bcomb = (bv @ proj_w + proj_b).astype(f32)               # [D]
    Mcat = np.concatenate([Mcomb, bcomb[None]], 0).astype(f32)  # [11, D]
    Gcat = (Mcat @ Mcat.T).astype(f32)                       # [11, 11]
    anchcat = np.concatenate([-2.0 * anch.T, Mcat.T], 1)     # [D, 21]
    na2cat = np.zeros((1, NCOLS), f32)
    na2cat[0, :A] = np.sum(anch * anch, axis=1)
    mask = np.triu(np.ones((A, A), f32), k=1)
    c = {
        "c_w1": w1.astype(bf16),
        "c_w2": np.ascontiguousarray(w2[:, 0].reshape(2, 128).T).astype(bf16),
        "c_b1": np.ascontiguousarray(b1.reshape(2, 128).T).astype(f32),
        "c_b2": np.full((128, 1), np.float32(b2[0]), f32),
        "c_anchcat": anchcat.astype(bf16),
        "c_na2cat": na2cat,
        "c_mcat": Mcat.astype(bf16),
        "c_gcat": Gcat.astype(bf16),
        "c_id16": np.eye(128, dtype=bf16),
        "c_id32": np.eye(128, dtype=f32),
        "c_onesr": np.ones((1, 128), f32),
        "c_onesc": np.ones((128, 1), f32),
        "c_ones10": np.ones((A, 1), f32),
        "c_mask": mask,
        "c_arT": np.ascontiguousarray(anchors.T).astype(f32),
        "c_arTm2": np.ascontiguousarray(-2.0 * anchors.T).astype(f32),
        "anchors": anchors.astype(f32),
        "center_loss_weight":
            np.full((1, 1), np.float32(center_loss_weight), f32),
    }
    return c


def kernel(**inputs):
    inputs = {k: np.asarray(v) for k, v in inputs.items()}
    consts = host_consts(
        inputs["anchors"], inputs["pos_embedding"], inputs["qkv_w"],
        inputs["qkv_b"], inputs["proj_w"], inputs["proj_b"], inputs["w1"],
        inputs["b1"], inputs["w2"], inputs["b2"],
        inputs["center_loss_weight"])
    feats = inputs["features"].astype(np.float32)
    in_maps = []
    for i in range(NCORES):
        m = dict(consts)
        m["features"] = np.ascontiguousarray(feats[i * BL:(i + 1) * BL])
        in_maps.append(m)

    if "nc" not in _CACHE:
        _CACHE["nc"] = build_nc()
    res = run_bass_kernel_spmd(_CACHE["nc"], in_maps,
                               core_ids=list(range(NCORES)))
    outs = res.results
    refined = np.concatenate([outs[i]["refined"] for i in range(NCORES)], 0)
    attention = np.concatenate([outs[i]["attention"] for i in range(NCORES)],
                               0)
    total = np.float32(sum(float(outs[i]["loss"][0, 0])
                           for i in range(NCORES)))
    return refined, total, attention
